# revision 17
# baseline (speedup 1.0000x reference)
"""3-hop GNN message passing (BPR/LightGCN style) on 8 Trainium2 NeuronCores.

Strategy: destination-sharded SpMMs with hop fusion. The 5 logical
segment-sum SpMMs run as 3 edge passes:

  A: g1i = iu(embed_user)            gather fp32 eu rows (256B descriptors)
  B: (g1u, g2u) = ui([ei | g1i])     one gather pass over an interleaved
                                     bf16 pair table (256B rows, two hops
                                     per descriptor), two matmuls per chunk
                                     sharing one one-hot S
  C: (g2i, g3i) = iu([g1u/3|g2u/4])  same trick; both matmuls accumulate
                                     into one PSUM so the result is already
                                     1/3*g2i + 1/4*g3i

Each pass shards edges across cores by destination row, gathers source
rows with bulk dma_gather (int16 indices, 25000-row table sections),
applies per-edge weights via one-hot matmuls (S[e,slot]=w_e) on the tensor
engine, accumulates per-block PSUM windows, and scatter-adds finished rows
to HBM. AllGathers (bf16) assemble the interleaved tables between passes.
The final combine out = ei + 1/2 g1i + 1/3 g2i + 1/4 g3i is absorbed:
part_C is pre-initialized with ei + 0.5*g1i and pass C deposits the rest.
Gathers and scatters round-robin over multiple SWDGE queues (descriptor
throughput saturates around two queues per direction).
"""
import sys
import os

sys.path.insert(0, "/opt/trn_rl_repo")

import numpy as np
import ml_dtypes

import concourse.bass as bass
import concourse.bacc as bacc
import concourse.tile as tile
from concourse import bass_utils, mybir

# problem constants (hardcoded per harness contract)
U, I, D, E = 100000, 50000, 64, 1250000
NCORES = 8
DU = U // NCORES           # users per core (dest shard for ui-SpMMs)
DI = I // NCORES           # items per core (dest shard for iu-SpMMs)
SEC = 25000                # table section rows (int16 gather index range)
NSEC_IU = U // SEC         # 4 sections of the user-side tables
NSEC_UI = I // SEC         # 2 sections of the item-side tables
W = 64                     # dest window (one-hot slot count)
K = 128                    # edges per chunk (PE contraction dim)
CPB = 3                    # chunks per block
BPS = 16                   # blocks per super-block (= 2 PSUM banks)
CH_SB = BPS * CPB          # 48 chunks per super-block
IDX_SB = CH_SB * K         # 6144 gather indices per super-block
ROWS_SB = BPS * W          # 1024 scatter rows per super-block

BF16 = ml_dtypes.bfloat16

_LAST_RESULTS = None       # run metadata for test harness


def _pack_type(dest, src, w, dshard, nsec):
    """Pack edges for one SpMM type (iu or ui) into the uniform SPMD layout.

    Returns dict with per-core arrays (idx16, slot, w, sidx16) and NSB.
    """
    dest = dest.astype(np.int64)
    src = src.astype(np.int64)
    w = w.astype(np.float32)
    core_of = dest // dshard
    sec_of = src // SEC

    # per (core, section): lists of (slot_stream, src_stream, w_stream, blocks)
    per_cs = {}
    nblk_max = 0
    for c in range(NCORES):
        for s in range(nsec):
            m = (core_of == c) & (sec_of == s)
            d = dest[m] - c * dshard
            sl = src[m] - s * SEC
            wv = w[m]
            order = np.argsort(d, kind="stable")
            d, sl, wv = d[order], sl[order], wv[order]
            # unique dests in order with counts
            ud, ustart, ucnt = np.unique(d, return_index=True, return_counts=True)
            blocks = []   # (base, span, nedges)
            cur_base = -1
            cur_cnt = 0
            slot_arr = np.empty(len(d), np.float32)
            blk_of_edge = np.empty(len(d), np.int64)
            cap = CPB * K
            for t in range(len(ud)):
                u, st, k = int(ud[t]), int(ustart[t]), int(ucnt[t])
                if cur_base < 0 or (u - cur_base) >= W or (cur_cnt + k) > cap:
                    if cur_base >= 0:
                        blocks.append((cur_base, cur_span, cur_cnt))
                    cur_base = u
                    cur_cnt = 0
                cur_span = u - cur_base + 1
                slot_arr[st:st + k] = u - cur_base
                blk_of_edge[st:st + k] = len(blocks)
                cur_cnt += k
            if cur_base >= 0:
                blocks.append((cur_base, cur_span, cur_cnt))
            per_cs[(c, s)] = (d, sl, wv, slot_arr, blk_of_edge, blocks)
            nblk_max = max(nblk_max, len(blocks))

    nsb = (nblk_max + BPS - 1) // BPS
    nblk = nsb * BPS

    # emit per-core uniform arrays
    ncols_ch = nsec * nsb * CH_SB          # chunk columns total
    out = {
        "idx16": np.zeros((NCORES, 128, nsec * nsb * IDX_SB // 16), np.int16),
        "slot": np.zeros((NCORES, 128, ncols_ch), np.float32),
        "w": np.zeros((NCORES, 128, ncols_ch), np.float32),
        "sidx16": np.zeros((NCORES, 128, nsec * nsb * ROWS_SB // 16), np.int16),
        "nsb": nsb,
    }
    trash = dshard  # rows [dshard, dshard+W) are trash
    for c in range(NCORES):
        for s in range(nsec):
            d, sl, wv, slot_arr, blk_of_edge, blocks = per_cs[(c, s)]
            # stream arrays padded to nblk blocks
            slots_total = nblk * CPB * K
            idx_st = np.zeros(slots_total, np.int16)
            slot_st = np.zeros(slots_total, np.float32)
            w_st = np.zeros(slots_total, np.float32)
            # place each block's edges at block*cap
            if len(d):
                # edges are already grouped by block in order
                blk_change = np.r_[True, blk_of_edge[1:] != blk_of_edge[:-1]]
                grp_start = np.maximum.accumulate(
                    np.where(blk_change, np.arange(len(d)), 0))
                edge_pos_in_blk = np.arange(len(d)) - grp_start
                pos = blk_of_edge * (CPB * K) + edge_pos_in_blk
                idx_st[pos] = sl.astype(np.int16)
                slot_st[pos] = slot_arr
                w_st[pos] = wv
            # wrap into device layouts
            base_col = s * nsb  # super-block offset for this section
            # gather idx: position i -> (row i%16, col i//16), tiled 8x
            idxw = idx_st.reshape(-1, 16).T  # [16, slots/16]
            cw0 = base_col * (IDX_SB // 16)
            out["idx16"][c][:, cw0:cw0 + idxw.shape[1]] = np.tile(idxw, (8, 1))
            # slot/w: chunk-major [128, cols]
            sm = slot_st.reshape(-1, K).T    # [128, ncols_cs]
            wm = w_st.reshape(-1, K).T
            cc0 = base_col * CH_SB
            out["slot"][c][:, cc0:cc0 + sm.shape[1]] = sm
            out["w"][c][:, cc0:cc0 + wm.shape[1]] = wm
            # scatter rows: per super-block 1024 rows; row n -> p=n%128, j=n//128
            srows = np.full(nblk * W, trash, np.int64)
            rr = np.arange(nblk * W)
            srows += rr % W  # default trash + r (unique per slot)
            for b, (base, span, cnt) in enumerate(blocks):
                r = np.arange(span)
                srows[b * W + r[:span]] = base + r[:span]
            # reorder into scatter enumeration: for each sb: n in [0,1024):
            # p = n%128, j = n//128; block_local = j + 8*(p>=64); r = p%64
            n = np.arange(nsb * ROWS_SB)
            p = n % 128
            j = (n // 128) % 8
            sb_i = n // ROWS_SB
            blk_l = sb_i * BPS + j + 8 * (p >= 64)
            r = p % 64
            sidx_strm = srows[blk_l * W + r].astype(np.int16)
            sw = sidx_strm.reshape(-1, 16).T
            sc0 = base_col * (ROWS_SB // 16)
            out["sidx16"][c][:, sc0:sc0 + sw.shape[1]] = np.tile(sw, (8, 1))
    return out


def _build_program(nsb_iu, nsb_ui):
    nq = int(os.environ.get("KERNEL_NQ", "4"))
    nc = bacc.Bacc("TRN2", target_bir_lowering=False, debug=False,
                   num_devices=NCORES, num_swdge_queues=nq)
    f32 = mybir.dt.float32
    bf16 = mybir.dt.bfloat16
    i16 = mybir.dt.int16

    t_eu = nc.dram_tensor("embed_user", [U, D], f32, kind="ExternalInput")
    tb_init = nc.dram_tensor("tb_init", [DI + W, 2 * D], bf16,
                             kind="ExternalInput")
    ei_slice = nc.dram_tensor("ei_slice", [DI, D], f32, kind="ExternalInput")
    iota_in = nc.dram_tensor("iota", [128, W], f32, kind="ExternalInput")

    iu_cols = NSEC_IU * nsb_iu
    ui_cols = NSEC_UI * nsb_ui
    iu_idx = nc.dram_tensor("iu_idx", [128, iu_cols * IDX_SB // 16], i16, kind="ExternalInput")
    iu_slot = nc.dram_tensor("iu_slot", [128, iu_cols * CH_SB], f32, kind="ExternalInput")
    iu_w = nc.dram_tensor("iu_w", [128, iu_cols * CH_SB], f32, kind="ExternalInput")
    iu_sidx = nc.dram_tensor("iu_sidx", [128, iu_cols * ROWS_SB // 16], i16, kind="ExternalInput")
    ui_idx = nc.dram_tensor("ui_idx", [128, ui_cols * IDX_SB // 16], i16, kind="ExternalInput")
    ui_slot = nc.dram_tensor("ui_slot", [128, ui_cols * CH_SB], f32, kind="ExternalInput")
    ui_w = nc.dram_tensor("ui_w", [128, ui_cols * CH_SB], f32, kind="ExternalInput")
    ui_sidx = nc.dram_tensor("ui_sidx", [128, ui_cols * ROWS_SB // 16], i16, kind="ExternalInput")

    out_ext = nc.dram_tensor("out", [DI, D], f32, kind="ExternalOutput")

    tb_local = nc.dram_tensor("tb_local", [DI + W, 2 * D], bf16, kind="Internal")
    table_B = nc.dram_tensor("table_B", [I, 2 * D], bf16, kind="Internal")
    tc_local = nc.dram_tensor("tc_local", [DU + W, 2 * D], bf16, kind="Internal")
    table_C = nc.dram_tensor("table_C", [U, 2 * D], bf16, kind="Internal")
    part_C = nc.dram_tensor("part_C", [DI + W, D], f32, kind="Internal")

    rg = [list(range(NCORES))]
    stage = int(os.environ.get("KERNEL_STAGE", "0"))
    sub = int(os.environ.get("KERNEL_SUB", "3"))
    repeat = int(os.environ.get("KERNEL_REPEAT", "1"))

    with tile.TileContext(nc) as tc:
        with (
            tc.tile_pool(name="const", bufs=1) as cpool,
            tc.tile_pool(name="sb", bufs=2) as sb,
            tc.tile_pool(name="gp", bufs=3) as gp,
            tc.tile_pool(name="spool", bufs=6) as spool,
            tc.tile_pool(name="stgp", bufs=3) as stgp,
            tc.tile_pool(name="psum", bufs=2, space="PSUM") as pp,
            tc.tile_pool(name="psum2", bufs=2, space="PSUM") as pp2,
        ):
            iota_t = cpool.tile([128, W], f32)
            nc.sync.dma_start(out=iota_t[:], in_=iota_in[:])

            # zero tile for clearing tc_local (bf16)
            ztb = cpool.tile([128, 48 * 2 * D], bf16)
            nc.vector.memset(ztb[:], 0.0)

            def zero_bf16(part, nrows, width):
                r0 = 0
                step = 128 * 48
                while r0 < nrows:
                    n = min(step, nrows - r0)
                    a = n // 128
                    if a >= 1:
                        nc.sync.dma_start(
                            out=part[r0:r0 + a * 128, :].rearrange(
                                "(a p) d -> p a d", p=128),
                            in_=ztb[:, :a * width].rearrange(
                                "p (a d) -> p a d", a=a),
                        )
                        r0 += a * 128
                    else:
                        nc.sync.dma_start(out=part[r0:r0 + n, :],
                                          in_=ztb[:n, :width])
                        r0 += n

            # tb_local <- tb_init (ei rows in cols 0:64, zeros elsewhere)
            nc.sync.dma_start(out=tb_local[:, :], in_=tb_init[:, :])
            zero_bf16(tc_local, DU + W, 2 * D)

            def spmm(kind, table, part, nsec, nsb, idx_in, slot_in, w_in,
                     sidx_in):
                """kind: 'A' (f32 single-table), 'B' (bf16 pair -> pair out),
                'C' (bf16 pair -> folded f32 out).

                Gathers span GSB=2 super-blocks; matmuls for B/C are one wide
                [128e -> 64slots x 128] op per chunk into [64, 512] PSUM tiles
                holding 4 wide block-columns each."""
                pair = kind in ("B", "C")
                gdt = bf16 if pair else f32
                gw = 2 * D if pair else D       # gathered row width (elems)
                sdt = bf16 if pair else f32     # S dtype must match gathered rows
                GSB = 2
                for s in range(nsec):
                    for isb0 in range(0, nsb, GSB):
                        nsb_g = min(GSB, nsb - isb0)   # superblocks this gather
                        g = s * nsb + isb0
                        qg = (g // GSB) % 2
                        idxt = sb.tile([128, GSB * IDX_SB // 16], i16, tag="idx")
                        nc.sync.dma_start(
                            out=idxt[:, :nsb_g * IDX_SB // 16],
                            in_=idx_in[:, g * (IDX_SB // 16):(g + nsb_g) * (IDX_SB // 16)])
                        slott = sb.tile([128, GSB * CH_SB], f32, tag="slot")
                        nc.sync.dma_start(
                            out=slott[:, :nsb_g * CH_SB],
                            in_=slot_in[:, g * CH_SB:(g + nsb_g) * CH_SB])
                        wt = sb.tile([128, GSB * CH_SB], f32, tag="w")
                        nc.sync.dma_start(
                            out=wt[:, :nsb_g * CH_SB],
                            in_=w_in[:, g * CH_SB:(g + nsb_g) * CH_SB])
                        sidxt = sb.tile([128, GSB * ROWS_SB // 16], i16, tag="sidx")
                        nc.sync.dma_start(
                            out=sidxt[:, :nsb_g * ROWS_SB // 16],
                            in_=sidx_in[:, g * (ROWS_SB // 16):(g + nsb_g) * (ROWS_SB // 16)])

                        gt = gp.tile([128, GSB * CH_SB * gw], gdt, tag="G")
                        nc.gpsimd.dma_gather(
                            out_ap=gt[:, :nsb_g * CH_SB * gw].rearrange(
                                "p (c d) -> p c d", c=nsb_g * CH_SB),
                            in_ap=table[s * SEC:(s + 1) * SEC, :],
                            idxs_ap=idxt[:, :nsb_g * IDX_SB // 16],
                            num_idxs=nsb_g * IDX_SB,
                            num_idxs_reg=nsb_g * IDX_SB,
                            elem_size=gw,
                            single_packet=False,
                            queue_num=qg,
                        )
                        if sub == 0:
                            probe_t = part_C if gdt == f32 else tc_local
                            nc.gpsimd.dma_start(out=probe_t[0:128, 0:D],
                                                in_=gt[:, 0:D])
                            continue
                        for isb_l in range(nsb_g):
                            _spmm_sb(kind, part, gt, slott, wt, sidxt,
                                     isb_l, qg, sdt, gw)

            def _spmm_sb(kind, part, gt, slott, wt, sidxt, isb_l, qg, sdt, gw):
                c0 = isb_l * CH_SB             # chunk offset in gt/slott/wt
                pair = kind in ("B", "C")
                # PSUM tiles: A: psA/psB narrow (8 blk x 64); B/C: 4 wide
                # tiles of 4 blocks x 128 cols
                if kind == "A":
                    psA = pp.tile([64, 512], f32, tag="psA")
                    psB = pp.tile([64, 512], f32, tag="psB")
                    pst = [psA, psB]
                else:
                    pst = [pp.tile([64, 512], f32, tag="psA", name="psA"),
                           pp.tile([64, 512], f32, tag="psB", name="psB"),
                           pp2.tile([64, 512], f32, tag="psC", name="psC"),
                           pp2.tile([64, 512], f32, tag="psD", name="psD")]
                for blk in range(BPS):
                    for ch in range(CPB):
                        ci = c0 + blk * CPB + ch
                        st = spool.tile([128, W], sdt, tag="S")
                        nc.vector.tensor_scalar(
                            out=st[:],
                            in0=iota_t[:],
                            scalar1=slott[:, ci:ci + 1],
                            scalar2=wt[:, ci:ci + 1],
                            op0=mybir.AluOpType.is_equal,
                            op1=mybir.AluOpType.mult,
                        )
                        if kind == "A":
                            p1 = pst[blk // 8]
                            col = blk % 8
                            nc.tensor.matmul(
                                out=p1[:, col * D:(col + 1) * D],
                                lhsT=st[:],
                                rhs=gt[:, ci * D:(ci + 1) * D],
                                start=(ch == 0),
                                stop=(ch == CPB - 1),
                            )
                        else:  # wide: [64 slots, 128] per block
                            p1 = pst[blk // 4]
                            col = blk % 4
                            nc.tensor.matmul(
                                out=p1[:, col * 2 * D:(col + 1) * 2 * D],
                                lhsT=st[:],
                                rhs=gt[:, ci * 2 * D:(ci + 1) * 2 * D],
                                start=(ch == 0),
                                stop=(ch == CPB - 1),
                            )
                # PSUM -> staging -> scatter
                sx = sidxt[:, isb_l * (ROWS_SB // 16):(isb_l + 1) * (ROWS_SB // 16)]
                qs = 2 + qg if nq >= 4 else qg
                if kind == "A":
                    # stg rows [0:64]=0 (ei half), [64:128]=g1i bf16
                    stg = stgp.tile([128, 8 * 2 * D], bf16, tag="stgA")
                    nc.vector.memset(stg[:], 0.0)
                    sv = stg[:].rearrange("p (c d) -> p c d", c=8)
                    nc.scalar.activation(
                        out=sv[0:64, :, D:2 * D], in_=pst[0][:].rearrange(
                            "p (c d) -> p c d", c=8),
                        func=mybir.ActivationFunctionType.Copy)
                    nc.scalar.activation(
                        out=sv[64:128, :, D:2 * D], in_=pst[1][:].rearrange(
                            "p (c d) -> p c d", c=8),
                        func=mybir.ActivationFunctionType.Copy)
                    if sub <= 1:
                        nc.gpsimd.dma_start(out=part[0:128, :],
                                            in_=stg[:, 0:2 * D])
                        return
                    nc.gpsimd.dma_scatter_add(
                        part[:], sv, sx, ROWS_SB, ROWS_SB, 2 * D,
                        queue_num=qs)
                elif kind == "B":
                    # stg rows = [g1u/3 | g2u/4] bf16; wide psum tiles hold
                    # 4 blocks x [g1u | g2u] each
                    stg = stgp.tile([128, 8 * 2 * D], bf16, tag="stgB")
                    sv = stg[:].rearrange("p (c d) -> p c d", c=8)
                    for half in range(2):          # stg partition half
                        po = half * 64
                        for jt in range(2):        # stg block cols j<4 / j>=4
                            t = pst[half * 2 + jt]
                            tv = t[:].rearrange("p (c d) -> p c d", c=4)
                            nc.scalar.activation(
                                out=sv[po:po + 64, jt * 4:(jt + 1) * 4, 0:D],
                                in_=tv[:, :, 0:D],
                                func=mybir.ActivationFunctionType.Copy,
                                scale=1.0 / 3.0)
                            nc.scalar.activation(
                                out=sv[po:po + 64, jt * 4:(jt + 1) * 4, D:2 * D],
                                in_=tv[:, :, D:2 * D],
                                func=mybir.ActivationFunctionType.Copy,
                                scale=0.25)
                    if sub <= 1:
                        nc.gpsimd.dma_start(out=part[0:128, :],
                                            in_=stg[:, 0:2 * D])
                        return
                    nc.gpsimd.dma_scatter_add(
                        part[:], sv, sx, ROWS_SB, ROWS_SB, 2 * D,
                        queue_num=qs)
                else:  # C: f32 folded rows (sum of wide halves)
                    stg = stgp.tile([128, 8 * D], f32, tag="stgC")
                    sv = stg[:].rearrange("p (c d) -> p c d", c=8)
                    for half in range(2):
                        po = half * 64
                        for jt in range(2):
                            t = pst[half * 2 + jt]
                            tmp = stgp.tile([64, 512], f32, tag="ctmp")
                            nc.scalar.activation(
                                out=tmp[:], in_=t[:],
                                func=mybir.ActivationFunctionType.Copy)
                            tv = tmp[:].rearrange("p (c d) -> p c d", c=4)
                            nc.vector.tensor_add(
                                sv[po:po + 64, jt * 4:(jt + 1) * 4, :],
                                tv[:, :, 0:D], tv[:, :, D:2 * D])
                    if sub <= 1:
                        nc.gpsimd.dma_start(out=part[0:128, :D],
                                            in_=stg[:, 0:D])
                        return
                    nc.gpsimd.dma_scatter_add(
                        part[:], sv, sx, ROWS_SB, ROWS_SB, D,
                        queue_num=qs)

            def init_part_c():
                # part_C[0:DI] = ei + 0.5 * g1i  (g1i = tb_local[:, D:2D] bf16)
                step = 128 * 24
                r0 = 0
                while r0 < DI:
                    n = min(step, DI - r0)
                    a = max(n // 128, 1)
                    n = a * 128 if n >= 128 else n
                    if n >= 128:
                        view = lambda t, w0, w1: t[r0:r0 + n, w0:w1].rearrange(
                            "(a p) d -> p a d", p=128)
                        eit = sb.tile([128, a * D], f32, tag="pc_e")
                        g1t = sb.tile([128, a * D], bf16, tag="pc_g")
                        acc = sb.tile([128, a * D], f32, tag="pc_a")
                        nc.sync.dma_start(
                            out=eit[:].rearrange("p (a d) -> p a d", a=a),
                            in_=view(ei_slice, 0, D))
                        nc.sync.dma_start(
                            out=g1t[:].rearrange("p (a d) -> p a d", a=a),
                            in_=view(tb_local, D, 2 * D))
                        nc.vector.tensor_scalar_mul(acc[:], g1t[:], 0.5)
                        nc.vector.tensor_add(acc[:], acc[:], eit[:])
                        nc.sync.dma_start(
                            out=view(part_C, 0, D),
                            in_=acc[:].rearrange("p (a d) -> p a d", a=a))
                        r0 += n
                    else:
                        eit = sb.tile([128, D], f32, tag="pc_e")
                        g1t = sb.tile([128, D], bf16, tag="pc_g")
                        acc = sb.tile([128, D], f32, tag="pc_a")
                        nc.sync.dma_start(out=eit[:n], in_=ei_slice[r0:r0 + n, :])
                        nc.sync.dma_start(out=g1t[:n], in_=tb_local[r0:r0 + n, D:2 * D])
                        nc.vector.tensor_scalar_mul(acc[:n], g1t[:n], 0.5)
                        nc.vector.tensor_add(acc[:n], acc[:n], eit[:n])
                        nc.sync.dma_start(out=part_C[r0:r0 + n, :], in_=acc[:n])
                        r0 += n
                # trash rows of part_C can hold garbage (never read), but
                # scatter-add needs them initialized to avoid NaN poisoning
                zf = cpool.tile([128, D], f32)
                nc.vector.memset(zf[:], 0.0)
                nc.sync.dma_start(out=part_C[DI:DI + W, :], in_=zf[:W])

            for _rep in range(repeat):
                # pass A: g1i into tb_local pair rows
                spmm("A", t_eu, tb_local, NSEC_IU, nsb_iu,
                     iu_idx, iu_slot, iu_w, iu_sidx)
                if stage == 1:
                    continue
                if stage != 2:
                    nc.gpsimd.collective_compute(
                        "AllGather", mybir.AluOpType.bypass, replica_groups=rg,
                        ins=[tb_local[0:DI, :]], outs=[table_B[:]])
                init_part_c()
                # pass B: (g1u/3 | g2u/4) into tc_local
                spmm("B", table_B, tc_local, NSEC_UI, nsb_ui,
                     ui_idx, ui_slot, ui_w, ui_sidx)
                if stage != 2:
                    nc.gpsimd.collective_compute(
                        "AllGather", mybir.AluOpType.bypass, replica_groups=rg,
                        ins=[tc_local[0:DU, :]], outs=[table_C[:]])
                # pass C: 1/3 g2i + 1/4 g3i scatter-added onto part_C
                if stage != 4:
                    spmm("C", table_C, part_C, NSEC_IU, nsb_iu,
                         iu_idx, iu_slot, iu_w, iu_sidx)

            # out = part_C[0:DI]
            step = 128 * 48
            r0 = 0
            while r0 < DI:
                n = min(step, DI - r0)
                a = max(n // 128, 1)
                if n >= 128:
                    n = a * 128
                    t = sb.tile([128, a * D], f32, tag="fin")
                    nc.sync.dma_start(
                        out=t[:].rearrange("p (a d) -> p a d", a=a),
                        in_=part_C[r0:r0 + n, :].rearrange(
                            "(a p) d -> p a d", p=128))
                    nc.sync.dma_start(
                        out=out_ext[r0:r0 + n, :].rearrange(
                            "(a p) d -> p a d", p=128),
                        in_=t[:].rearrange("p (a d) -> p a d", a=a))
                else:
                    t = sb.tile([128, D], f32, tag="fin")
                    nc.sync.dma_start(out=t[:n], in_=part_C[r0:r0 + n, :])
                    nc.sync.dma_start(out=out_ext[r0:r0 + n, :], in_=t[:n])
                r0 += n

    nc.compile()
    return nc


def _make_in_maps(inputs, iu, ui):
    embed_user = np.asarray(inputs["embed_user"], np.float32)
    embed_item = np.asarray(inputs["embed_item"], np.float32)
    iota = np.broadcast_to(np.arange(W, dtype=np.float32), (128, W)).copy()
    in_maps = []
    for c in range(NCORES):
        ei_sl = np.ascontiguousarray(embed_item[c * DI:(c + 1) * DI])
        tb0 = np.zeros((DI + W, 2 * D), BF16)
        tb0[:DI, :D] = ei_sl.astype(BF16)
        in_maps.append({
            "embed_user": embed_user,
            "tb_init": tb0,
            "ei_slice": ei_sl,
            "iota": iota,
            "iu_idx": iu["idx16"][c], "iu_slot": iu["slot"][c],
            "iu_w": iu["w"][c], "iu_sidx": iu["sidx16"][c],
            "ui_idx": ui["idx16"][c], "ui_slot": ui["slot"][c],
            "ui_w": ui["w"][c], "ui_sidx": ui["sidx16"][c],
        })
    return in_maps


def kernel(embed_user, embed_item, edge_vals, u_idx, i_idx):
    global _LAST_RESULTS
    inputs = {
        "embed_user": np.asarray(embed_user, np.float32),
        "embed_item": np.asarray(embed_item, np.float32),
    }
    edge_vals = np.asarray(edge_vals, np.float32)
    u_idx = np.asarray(u_idx).astype(np.int64)
    i_idx = np.asarray(i_idx).astype(np.int64)

    # pack both SpMM edge types
    iu = _pack_type(i_idx, u_idx, edge_vals, DI, NSEC_IU)   # dest=item, src=user
    ui = _pack_type(u_idx, i_idx, edge_vals, DU, NSEC_UI)   # dest=user, src=item

    nc = _build_program(iu["nsb"], ui["nsb"])
    in_maps = _make_in_maps(inputs, iu, ui)

    trace = bool(int(os.environ.get("KERNEL_TRACE", "0")))
    res = bass_utils.run_bass_kernel_spmd(
        nc, in_maps, core_ids=list(range(NCORES)), trace=trace)
    _LAST_RESULTS = res
    out = np.concatenate([res.results[c]["out"] for c in range(NCORES)], axis=0)
    return out
